# revision 84
# baseline (speedup 1.0000x reference)
"""Multi-head attention (RoPE + u-bias + bool mask) Trainium2 Bass kernel.

Contract: kernel(**inputs) takes FULL unsharded inputs (see shapes below),
shards batch across 8 NeuronCores (data parallel), runs one Bass/Tile
program per core, and gathers the full output.

Hardcoded problem shapes:
  query/key/value: (8, 1024, 1024) f32, mask: (8, 1024, 1024) bool,
  Wq/Wk/Wv/Wo: (1024, 1024) f32, bq/bk/bv/bo: (1024,) f32,
  u_bias: (16, 64) f32.  Output: (8, 1024, 1024) f32.

v4 (sim 254us vs v3's 314us, PE 92% occupied; rel err 0.0093 vs 0.0127):
one interleaved schedule so the exp stream (ACT-bound, ~18us per head-pair
j) overlaps the projection matmuls instead of serializing after them.
Order: V-projection (B) first, then per j the QK-projection+rope (A),
scores, exp, mask-mult, ctx and normalize units run as one software-
pipelined stream -- A(j+1) is slotted into j's unit stream, B(st4..7)
into j0's, D(st0) k<7 partials into j7's empty slots -- out-projection
(D) last.  RoPE reads the raw QK PSUM accumulator directly (fp32, and
PSUM operands are exempt from the verifier's equal-SB-base rule):
t1=(raw+b)*cos, t2f=(raw+b)*sin_signflipped, and the p^32 partner swap
(host de-interleaves W rows per head to [evens, odds]) runs as partition-
offset SBUF->SBUF DMAs on the idle SP queue -- no permutation matmuls, no
separate evacuation.  Scores are emitted as (hi=0, hi=1) row-tile pairs
(K=64, base partitions 0/64) so the PE array runs both heads concurrently
on HW.  Softmax denominators come free from the ones-column of augmented
V; reciprocal reads the PSUM ones-row directly; normalization broadcasts
1/den via two K=1 ones-matmuls per half.  Inputs stream on both HWDGE
queues (SP + ACT) in consumption order; elementwise work is balanced
across DVE and GPSIMD (K_EM_DVE) with ACT kept exp-only mid-stream (a
Copy/Identity op on ACT costs a ~1.3us activation-table reload vs Exp).
"""

import os
import sys

if "/opt/trn_rl_repo" not in sys.path:
    sys.path.insert(0, "/opt/trn_rl_repo")

from contextlib import ExitStack

import ml_dtypes
import numpy as np

import concourse.bass as bass
from concourse import bacc
import concourse.tile as tile
from concourse import mybir
from concourse.bass_utils import run_bass_kernel_spmd

B, S, D, H, Dh = 8, 1024, 1024, 16, 64
P = 128
NT = D // P  # 8 partition-tiles along d
ST = S // P  # 8 tiles along s/t
HF = S // 2  # 512 = matmul moving chunk / PSUM bank width (fp32)
FP = mybir.dt.float32
BF = mybir.dt.bfloat16
ROPE_BASE = 10000.0
AF = mybir.ActivationFunctionType
ALU = mybir.AluOpType

N_CORES = 8
STAGES = int(os.environ.get("K_STAGES", "5"))
# of the 32 (tt,c,hi) mask-mult units per j, how many go to DVE (rest GPSIMD)
EM_DVE = int(os.environ.get("K_EM_DVE", "16"))
CTX_LAG = int(os.environ.get("K_L", "8"))  # units ctx trails scores by

# column offsets inside the packed mega input tensor
_SIZES = [
    ("xq", NT * S), ("xk", NT * S), ("xv", ST * NT * P),
    ("wq", NT * D), ("wk", NT * D), ("wv", NT * D), ("wo", NT * D),
    ("mask", ST * S), ("cs", 2 * S), ("smalls", 24),
    ("rows", D + 3 * P),
]
OFF = {}
_o = 0
for _n, _s in _SIZES:
    OFF[_n] = _o
    _o += _s
MEGA_COLS = _o


def build_nc():
    nc = bacc.Bacc("TRN2", target_bir_lowering=False, debug=False)

    # Single mega input: every tensor packed into one [P, MEGA_COLS] bf16
    # DRAM tensor (per-input dispatch overhead through the PJRT tunnel is
    # ~70us/tensor, so one input instead of 14 dominates the bench time).
    mega = nc.dram_tensor("mega", [P, MEGA_COLS], BF, kind="ExternalInput").ap()
    xq_d = mega[:, OFF["xq"] : OFF["xq"] + NT * S]
    xk_d = mega[:, OFF["xk"] : OFF["xk"] + NT * S]
    xv_d = mega[:, OFF["xv"] : OFF["xv"] + ST * NT * P]  # st-major layout
    wq_d = mega[:, OFF["wq"] : OFF["wq"] + NT * D]
    wk_d = mega[:, OFF["wk"] : OFF["wk"] + NT * D]
    wv_d = mega[:, OFF["wv"] : OFF["wv"] + NT * D]
    wo_d = mega[:, OFF["wo"] : OFF["wo"] + NT * D]
    mask_d = mega[:, OFF["mask"] : OFF["mask"] + ST * S]
    # cs[:, 0:S] = cos table, cs[:, S:2S] = sign-folded sin table
    cs_d = mega[:, OFF["cs"] : OFF["cs"] + 2 * S]
    # smalls[:, 0:8]=u cols, 8:16=bq, 16:24=bk (de-interleaved d order)
    smalls_d = mega[:, OFF["smalls"] : OFF["smalls"] + 24]
    # rows[0, 0:D]=bvo (= Wo@bv + bo, the V-bias folded through attention
    # since softmax rows sum to 1), D:D+P=ones, then colones0, colones1
    rows_d = mega[0:1, OFF["rows"] : OFF["rows"] + D + 3 * P]
    out_d = nc.dram_tensor("out", [S, D], FP, kind="ExternalOutput").ap()

    with tile.TileContext(nc) as tc, ExitStack() as ctx:
        persist = ctx.enter_context(tc.tile_pool(name="persist", bufs=1))
        psum = ctx.enter_context(tc.tile_pool(name="ps", bufs=1, space="PSUM"))
        work = ctx.enter_context(tc.tile_pool(name="work", bufs=1))

        # ---- persistent constants / state ----
        smalls_sb = persist.tile([P, 24], BF)
        ucols = smalls_sb[:, 0:8]
        # scalar operands for the rope stt ops in fp32
        smalls_f = persist.tile([P, 16], FP)
        bqcols = smalls_f[:, 0:8]
        bkcols = smalls_f[:, 8:16]
        rows_sb = persist.tile([1, D + 3 * P], BF)
        bvorow = rows_sb[:, 0:D]
        ones_row = rows_sb[:, D : D + P]
        # [65,128] selector for the 1/den broadcast matmul: row 0 -> head 2j
        # columns, row 64 -> head 2j+1 columns, middle rows zero
        sel65 = persist.tile([65, P], BF)
        nc.gpsimd.memset(sel65[:], 0.0)
        nc.gpsimd.memset(sel65[0:1, 0:Dh], 1.0)
        nc.gpsimd.memset(sel65[64:65, Dh:P], 1.0)
        # persistent [65,S] carrier for the two reciprocal rows (partitions
        # 0 and 64 are both legal engine start partitions); middle rows
        # zeroed once so the K=65 matmul contracts them to nothing
        rec01 = persist.tile([65, S], BF)
        nc.gpsimd.memset(rec01[:], 0.0)
        cs_sb = persist.tile([P, 2 * S], BF)
        cos_sb = cs_sb[:, 0:S]
        sin_sb = cs_sb[:, S : 2 * S]
        mask_sb = persist.tile([P, ST * S], BF)

        # V augmented with ones column: [p, tt, h, 0:64]=V, [.,.,.,64]=1
        vaug = persist.tile([P, ST * H * (Dh + 1)], BF)
        vaug_v = vaug[:].rearrange("p (st h c) -> p st h c", st=ST, h=H)
        ctxu = persist.tile([P, NT * S], BF)  # ctx^T (normalized in place)
        # roped Q^T + u / roped K^T, one [P, S] tile per j (3 slots in flight)
        qbt = {}
        kbt = {}

        nc.gpsimd.memset(vaug_v[:, :, :, Dh : Dh + 1], 1.0)

        # ---- DMAs: two HWDGE queues (SP + ACT). SP streams wv/xv-rest/
        # wk/xk then j0's rope-swap chunks; ACT takes xv0 + consts + wq/xq
        # + mask so B and A(j0) start as early as possible ----

        # B-path inputs on the SP DMA queue, A-path weights in parallel on
        # the second (Activation) HWDGE queue
        pB_cm = tc.tile_pool(name="bpool", bufs=1)
        pB = pB_cm.__enter__()
        wv_sb = pB.tile([P, NT * D], BF, tag="wv")
        # chunked so B(st0)'s k-loop starts while the rest streams in
        xv_sb = pB.tile([P, ST * NT * P], BF, tag="xv")
        nc.sync.dma_start(wv_sb[:, 0 : 2 * D], wv_d[:, 0 : 2 * D])
        nc.scalar.dma_start(xv_sb[:, 0 : NT * P], xv_d[:, 0 : NT * P])
        nc.scalar.dma_start(rows_sb[:], rows_d[:])
        nc.scalar.dma_start(smalls_sb[:], smalls_d[:])
        nc.vector.tensor_copy(smalls_f[:], smalls_sb[:, 8:24])
        nc.scalar.dma_start(cs_sb[:], cs_d[:])
        for q in range(1, 4):
            nc.sync.dma_start(
                wv_sb[:, q * 2 * D : (q + 1) * 2 * D],
                wv_d[:, q * 2 * D : (q + 1) * 2 * D],
            )
        for st in range(1, 4):
            nc.sync.dma_start(
                xv_sb[:, st * NT * P : (st + 1) * NT * P],
                xv_d[:, st * NT * P : (st + 1) * NT * P],
            )
        wq_sb = work.tile([P, NT * D], BF, tag="wq")
        nc.scalar.dma_start(wq_sb[:], wq_d[:])
        xq_sb = work.tile([P, NT * S], BF, tag="xq")
        nc.scalar.dma_start(xq_sb[:], xq_d[:])
        wk_sb = work.tile([P, NT * D], BF, tag="wk")
        nc.sync.dma_start(wk_sb[:], wk_d[:])
        xk_sb = work.tile([P, NT * S], BF, tag="xk")
        nc.sync.dma_start(xk_sb[:], xk_d[:])
        # "late" pool opens after bpool closes (reuses the wv/xv SBUF);
        # emitters reach it via this cell
        late_cell = {}

        # ======== stage B: V projection into vaug ========
        def emit_B(st):
            for c in range(2):
                vp = psum.tile([P, HF], FP, tag="acc", bufs=2, name=f"vp{st}_{c}")
                for k in range(NT):
                    nc.tensor.matmul(
                        vp[:],
                        xv_sb[:, st * NT * P + k * P : st * NT * P + (k + 1) * P],
                        wv_sb[:, k * D + c * HF : k * D + (c + 1) * HF],
                        start=(k == 0),
                        stop=(k == NT - 1),
                    )
                # DVE is idle during B; ACT's queue is busy issuing the
                # A-path DMAs at the head
                nc.vector.tensor_copy(
                    vaug_v[:, st, c * 8 : (c + 1) * 8, 0:Dh],
                    vp[:].rearrange("p (h c) -> p h c", h=8),
                )

        # ======== interleaved per-j stream: A (QK proj + rope) + C units ====
        # A-group g of j: g = x*2 + c with x in (q=0, k=1), c in (0, 1)
        a_state = {}

        def emit_A_raw(j, g):
            # group order (q,c0), (k,c0), (q,c1), (k,c1): the c0 tails land
            # first, which is all scores(u0/u1) need
            c, x = divmod(g, 2)
            x_sb, w_sb = (xq_sb, wq_sb) if x == 0 else (xk_sb, wk_sb)
            raw = psum.tile([P, HF], FP, tag="acc", bufs=2, name=f"raw{j}_{g}")
            for k in range(NT):
                nc.tensor.matmul(
                    raw[:],
                    w_sb[:, k * D + j * P : k * D + (j + 1) * P],
                    x_sb[:, k * S + c * HF : k * S + (c + 1) * HF],
                    start=(k == 0),
                    stop=(k == NT - 1),
                )
            a_state[(j, g)] = raw

        def emit_A_tail(j, g):
            c, x = divmod(g, 2)
            chalf = slice(c * HF, (c + 1) * HF)
            raw = a_state.pop((j, g))
            bcol = (bqcols if x == 0 else bkcols)[:, j : j + 1]
            # rope fused against the PSUM accumulator (PSUM operand is exempt
            # from the equal-SB-base-partition rule):
            #   t1[p]  = (raw[p] + b[p]) * cos[p]
            #   t2f[p] = (raw[p] + b[p]) * sinsw[p]   (sign-flipped sin table
            #            = the p^32 partner's coefficient)
            # then the p^32 partner swap runs as 4 partition-offset
            # SBUF->SBUF DMAs on the idle SP queue
            t1 = work.tile([P, HF], BF, tag="t1", bufs=2)
            nc.vector.scalar_tensor_tensor(
                t1[:], raw[:], bcol, cos_sb[:, chalf], op0=ALU.add, op1=ALU.mult
            )
            t2f = work.tile([P, HF], BF, tag="t2f", bufs=2)
            nc.vector.scalar_tensor_tensor(
                t2f[:], raw[:], bcol, sin_sb[:, chalf], op0=ALU.add, op1=ALU.mult
            )
            t2 = work.tile([P, HF], BF, tag="t2", bufs=2)
            for b in (0, 64):
                nc.sync.dma_start(t2[b : b + 32, :], t2f[b + 32 : b + 64, :])
                nc.sync.dma_start(t2[b + 32 : b + 64, :], t2f[b : b + 32, :])
            dst = qbt if x == 0 else kbt
            if j not in dst:
                dst[j] = work.tile(
                    [P, S], BF, tag=("qbt" if x == 0 else "kbt"), bufs=3,
                    name=f"{'qk'[x]}bt{j}",
                )
            dslice = dst[j][:, c * HF : (c + 1) * HF]
            if x == 0:
                # stt (u-bias scalar) only exists on DVE
                nc.vector.scalar_tensor_tensor(
                    dslice, t1[:], ucols[:, j : j + 1], t2[:],
                    op0=ALU.add, op1=ALU.add,
                )
            else:
                nc.gpsimd.tensor_tensor(dslice, t1[:], t2[:], op=ALU.add)

        # C unit u of j: u = tt*4 + c*2 + hi (hi innermost: row-tile pairs)
        em_tiles = {}
        cps_tiles = {}

        def emit_scores(j, u):
            tt, r = divmod(u, 4)
            c, hi = divmod(r, 2)
            half = hi * Dh
            sps = psum.tile([P, HF], FP, tag="sps", bufs=2, name=f"sps{j}_{u}")
            nc.tensor.matmul(
                sps[:],
                kbt[j][half : half + Dh, tt * P : (tt + 1) * P],
                qbt[j][half : half + Dh, c * HF : (c + 1) * HF],
                start=True,
                stop=True,
            )
            et = work.tile([P, HF], BF, tag="et", bufs=3)
            nc.scalar.activation(et[:], sps[:], AF.Exp, scale=0.125)
            em = work.tile([P, HF], BF, tag="em", bufs=CTX_LAG + 2)
            # spread the DVE/GPSIMD split evenly across the unit stream
            uu = u % 32
            eng = (
                nc.vector
                if (uu * EM_DVE) // 32 != ((uu + 1) * EM_DVE) // 32
                else nc.gpsimd
            )
            eng.tensor_tensor(
                em[:], et[:],
                mask_sb[:, tt * S + c * HF : tt * S + (c + 1) * HF],
                op=ALU.mult,
            )
            em_tiles[(j, u)] = em

        def emit_ctx(j, u):
            tt, r = divmod(u, 4)
            c, hi = divmod(r, 2)
            h = 2 * j + hi
            if tt == 0 and c == 0:
                cps_tiles[(j, hi)] = psum.tile(
                    [Dh + 1, S], FP, tag="cps", bufs=2, name=f"cps{j}_{hi}"
                )
            em = em_tiles.pop((j, u))
            nc.tensor.matmul(
                cps_tiles[(j, hi)][:, c * HF : (c + 1) * HF],
                vaug_v[:, tt, h, :],
                em[:],
                start=(tt == 0),
                stop=(tt == ST - 1),
            )

        rec_tiles = {}
        d_pre = {}

        def emit_C2_a(j):
            # slot-freeing half: reciprocals read the PSUM ones-row directly
            # (partition 64 -> 0), ctx evacuated to SBUF, cps banks released
            cp0 = cps_tiles.pop((j, 0))
            cp1 = cps_tiles.pop((j, 1))
            with nc.allow_low_precision(reason="bf16 1/den, ~0.4% quant"):
                nc.vector.reciprocal(rec01[0:1, :], cp0[Dh : Dh + 1, :])
                nc.vector.reciprocal(rec01[64:65, :], cp1[Dh : Dh + 1, :])
            # evacuate ctx (heads 2j -> partitions 0:64, 2j+1 -> 64:128);
            # hi=1 on ACT (Copy is table-free, no Exp thrash)
            nc.vector.tensor_copy(ctxu[0:Dh, j * S : (j + 1) * S], cp0[0:Dh, :])
            nc.scalar.copy(ctxu[Dh:P, j * S : (j + 1) * S], cp1[0:Dh, :])

        def emit_C2_b(j, rb_tag="acc"):
            # deferred half: broadcast 1/den across partitions via one K=65
            # matmul per half, then normalize in place (long dep slack by now)
            for c in range(2):
                chalf = slice(c * HF, (c + 1) * HF)
                # acc tag: A(j+1) raws are done by u26, no contention with
                # the scores stream on the sps slots
                rb = psum.tile([P, HF], FP, tag=rb_tag, bufs=2, name=f"rb{j}_{c}")
                nc.tensor.matmul(rb[:], sel65[:], rec01[:, chalf], start=True, stop=True)
                nc.vector.tensor_tensor(
                    ctxu[:, j * S + c * HF : j * S + (c + 1) * HF],
                    ctxu[:, j * S + c * HF : j * S + (c + 1) * HF],
                    rb[:],
                    op=ALU.mult,
                )

        # ---- emission schedule ----
        for st in range(4):
            emit_B(st)

        if STAGES >= 4:
            NU = 4 * ST  # 32 units per j
            # j>=1 slots: next j's projections early, C2 split around them
            A_RAW_SLOT = {8: 0, 12: 1, 16: 2, 20: 3}
            A_TAIL_SLOT = {10: 0, 14: 1, 18: 2, 22: 3}
            # j==0 slots: remaining B groups early (ctx(tt) needs vaug(tt)),
            # A(1) after
            B_SLOT_J0 = {8: 4, 12: 5, 14: 6, 16: 7}
            A_RAW_SLOT_J0 = {20: 0, 22: 1, 24: 2, 26: 3}
            A_TAIL_SLOT_J0 = {21: 0, 23: 1, 25: 2, 27: 3}
            # mask rides the ACT queue (idle after xq) so j0's rope-swap
            # DMAs get the SP queue right after xk
            for tt in range(ST):
                nc.scalar.dma_start(
                    mask_sb[:, tt * S : (tt + 1) * S],
                    mask_d[:, tt * S : (tt + 1) * S],
                )
            for g in range(4):  # j=0 A-groups up front, 1-deep pipelined
                emit_A_raw(0, g)
                if g >= 1:
                    emit_A_tail(0, g - 1)
            emit_A_tail(0, 3)
            # rest of xv after j0's swaps (needed only at j0's B slots)
            for st in range(4, ST):
                nc.sync.dma_start(
                    xv_sb[:, st * NT * P : (st + 1) * NT * P],
                    xv_d[:, st * NT * P : (st + 1) * NT * P],
                )
            for j in range(NT):
                for u in range(NU):
                    emit_scores(j, u)
                    if u >= CTX_LAG:
                        emit_ctx(j, u - CTX_LAG)
                    if j >= 1:
                        if u == 2:
                            emit_C2_a(j - 1)
                        elif u == (6 if j == NT - 1 else 26):
                            # last j: C2_b early so its acc slots free up for
                            # the D(st0) partial accumulations below
                            emit_C2_b(j - 1)
                        if j + 1 < NT:
                            if u in A_RAW_SLOT:
                                emit_A_raw(j + 1, A_RAW_SLOT[u])
                            elif u in A_TAIL_SLOT:
                                emit_A_tail(j + 1, A_TAIL_SLOT[u])
                        elif STAGES >= 5 and u in (10, 14):
                            # fill j7's empty A-slots: D(st0) k0..6 partials
                            # (ctxu j0..j6 are final by now)
                            c = 0 if u == 10 else 1
                            op = psum.tile(
                                [P, HF], FP, tag="acc", bufs=2, name=f"op0_{c}"
                            )
                            d_pre[c] = op
                            for k in range(NT - 1):
                                nc.tensor.matmul(
                                    op[:],
                                    ctxu[:, k * S : k * S + P],
                                    wo_sb[:, k * D + c * HF : k * D + (c + 1) * HF],
                                    start=(k == 0),
                                    stop=False,
                                )
                    else:
                        if u in B_SLOT_J0:
                            emit_B(B_SLOT_J0[u])
                        elif u in A_RAW_SLOT_J0:
                            emit_A_raw(1, A_RAW_SLOT_J0[u])
                        elif u in A_TAIL_SLOT_J0:
                            emit_A_tail(1, A_TAIL_SLOT_J0[u])
                for u in range(NU - CTX_LAG, NU):
                    emit_ctx(j, u)
                if j == 0:
                    # wv/xv done: free bpool, open late (reuses its SBUF)
                    pB_cm.__exit__(None, None, None)
                    late = ctx.enter_context(tc.tile_pool(name="late", bufs=1))
                    late_cell["p"] = late
                    wo_sb = late.tile([P, NT * D], BF, tag="wo")
                    nc.sync.dma_start(wo_sb[:], wo_d[:])
            emit_C2_a(NT - 1)
            # sps tag: the acc slots are held by the D(st0) partials
            emit_C2_b(NT - 1, rb_tag="sps")
        else:
            for st in range(4, ST):
                emit_B(st)
            pB_cm.__exit__(None, None, None)
            late = ctx.enter_context(tc.tile_pool(name="late", bufs=1))
            late_cell["p"] = late
            wo_sb = late.tile([P, NT * D], BF, tag="wo")
            nc.sync.dma_start(wo_sb[:], wo_d[:])

        # ---- stage D: output projection ----
        if STAGES >= 5:
            for st in range(ST):
                for c in range(2):
                    if st == 0 and c in d_pre:
                        # k0..6 already accumulated during j7's stream
                        op = d_pre.pop(c)
                        ks = range(NT - 1, NT)
                    else:
                        op = psum.tile(
                            [P, HF], FP, tag="acc", bufs=2, name=f"op{st}_{c}"
                        )
                        ks = range(NT)
                    for k in ks:
                        nc.tensor.matmul(
                            op[:],
                            ctxu[:, k * S + st * P : k * S + (st + 1) * P],
                            wo_sb[:, k * D + c * HF : k * D + (c + 1) * HF],
                            start=(k == 0),
                            stop=False,
                        )
                    nc.tensor.matmul(
                        op[:],
                        ones_row,
                        bvorow[:, c * HF : (c + 1) * HF],
                        start=False,
                        stop=True,
                    )
                    # DVE is idle during D; ACT may still be draining exp
                    ot = late.tile([P, HF], FP, tag="ot", bufs=3)
                    nc.vector.tensor_copy(ot[:], op[:])
                    nc.sync.dma_start(
                        out_d[st * P : (st + 1) * P, c * HF : (c + 1) * HF], ot[:]
                    )

    nc.compile()
    return nc


def _to_sb(m):
    """[NT*P, X] -> [P, NT*X] SBUF layout (partition p holds rows p, P+p, ...)."""
    r, x = m.shape
    return np.ascontiguousarray(
        m.reshape(NT, P, x).transpose(1, 0, 2).reshape(P, NT * x)
    )


# de-interleave permutation: within each head's 64 output dims, evens first
_PERM = np.concatenate(
    [64 * h + np.concatenate([np.arange(0, 64, 2), np.arange(1, 64, 2)]) for h in range(H)]
)


def _host_consts():
    inv_freq = 1.0 / (ROPE_BASE ** (np.arange(0, Dh, 2, dtype=np.float64) / Dh))
    # de-interleaved layout: partition p (within a 64-block) holds dim
    # evens[p] for p<32, odds[p-32] for p>=32; both use freq index p%32
    freqs = np.arange(S, dtype=np.float64)[:, None] * inv_freq[None, :]  # [S, 32]
    cosf = np.cos(freqs)  # [S, 32]
    sinf = np.sin(freqs)
    costab = np.empty((P, S), np.float32)
    sintab = np.empty((P, S), np.float32)
    for p in range(P):
        f = p % 32
        costab[p, :] = cosf[:, f]
        # table holds the coefficient the p^32 PARTNER needs (t2f is computed
        # at the source partition, then DMA-swapped): partner of an even slot
        # is odd (sgn +1), partner of an odd slot is even (sgn -1)
        sgn = 1.0 if (p % 64) < 32 else -1.0
        sintab[p, :] = sgn * sinf[:, f]
    return costab, sintab


_CONSTS = {}


def host_in_maps(query, key, value, mask, Wq, bq, Wk, bk, Wv, bv, u_bias, Wo, bo):
    bf = ml_dtypes.bfloat16
    if not _CONSTS:
        costab, sintab = _host_consts()
        _CONSTS["cs"] = np.concatenate([costab, sintab], axis=1)
    u = np.asarray(u_bias, np.float32).reshape(H * Dh)[_PERM].reshape(H, Dh)
    bq_p = np.asarray(bq, np.float32)[_PERM]
    bk_p = np.asarray(bk, np.float32)[_PERM]
    smalls = np.zeros((P, 24), np.float32)
    for j in range(NT):
        smalls[:, j] = np.concatenate([u[2 * j], u[2 * j + 1]])
    smalls[:, 8:16] = bq_p.reshape(NT, P).T
    smalls[:, 16:24] = bk_p.reshape(NT, P).T
    rows = np.zeros((P, D + 3 * P), np.float32)
    colones0 = np.zeros(P, np.float32)
    colones0[0:Dh] = 1.0
    colones1 = np.zeros(P, np.float32)
    colones1[Dh:P] = 1.0
    # attn rows sum to 1, so attn@(V+bv) = attn@V + bv; fold bv through the
    # out projection: bvo = Wo@bv + bo
    bvo = np.asarray(Wo, np.float32) @ np.asarray(bv, np.float32) + np.asarray(
        bo, np.float32
    )
    rows[0] = np.concatenate([bvo, np.ones(P, np.float32), colones0, colones1])

    def pack(pieces):
        m = np.empty((P, MEGA_COLS), bf)
        for name, arr in pieces.items():
            o = OFF[name]
            m[:, o : o + arr.shape[1]] = arr.astype(bf)
        return m

    def xv_layout(v):
        # [P, k*S + s] -> [P, st*(NT*P) + k*P + ss] (st-major)
        a = _to_sb(v)  # [P, NT*S]
        a = a.reshape(P, NT, ST, P).transpose(0, 2, 1, 3).reshape(P, ST * NT * P)
        return np.ascontiguousarray(a)

    shared = dict(
        wq=_to_sb(np.asarray(Wq, np.float32).T[:, _PERM].astype(bf)),
        wk=_to_sb(np.asarray(Wk, np.float32).T[:, _PERM].astype(bf)),
        wv=_to_sb(np.asarray(Wv, np.float32).T.astype(bf)),
        wo=_to_sb(np.asarray(Wo, np.float32).T.astype(bf)),
        cs=_CONSTS["cs"],
        smalls=smalls,
        rows=rows,
    )
    in_maps = []
    for b in range(N_CORES):
        pieces = dict(shared)
        pieces["xq"] = _to_sb(np.asarray(query[b], np.float32).T.astype(bf))
        pieces["xk"] = _to_sb(np.asarray(key[b], np.float32).T.astype(bf))
        pieces["xv"] = xv_layout(np.asarray(value[b], np.float32).T.astype(bf))
        pieces["mask"] = _to_sb((~np.asarray(mask[b], bool)).T.astype(bf))
        in_maps.append(dict(mega=pack(pieces)))
    return in_maps


_CACHED = {}


def kernel(query, key, value, mask, Wq, bq, Wk, bk, Wv, bv, u_bias, Wo, bo):
    if "nc" not in _CACHED:
        _CACHED["nc"] = build_nc()
    nc = _CACHED["nc"]
    in_maps = host_in_maps(
        query, key, value, mask, Wq, bq, Wk, bk, Wv, bv, u_bias, Wo, bo
    )
    res = run_bass_kernel_spmd(nc, in_maps, list(range(N_CORES)))
    return np.stack([res.results[b]["out"] for b in range(N_CORES)], axis=0)


# revision 101
# speedup vs baseline: 1.3615x; 1.3615x over previous
"""Multi-head attention (RoPE + u-bias + bool mask) Trainium2 Bass kernel.

Contract: kernel(**inputs) takes FULL unsharded inputs (see shapes below),
shards batch across 8 NeuronCores (data parallel), runs one Bass/Tile
program per core, and gathers the full output.

Hardcoded problem shapes:
  query/key/value: (8, 1024, 1024) f32, mask: (8, 1024, 1024) bool,
  Wq/Wk/Wv/Wo: (1024, 1024) f32, bq/bk/bv/bo: (1024,) f32,
  u_bias: (16, 64) f32.  Output: (8, 1024, 1024) f32.

v4 (sim 247us vs v3's 314us, PE busy 224us at 90% occupancy; rel err
0.0093 vs 0.0127):
one interleaved schedule so the exp stream (ACT-bound, ~18us per head-pair
j) overlaps the projection matmuls instead of serializing after them.
Order: V-projection (B) first, then per j the QK-projection+rope (A),
scores, exp, mask-mult, ctx and normalize units run as one software-
pipelined stream -- A(j+1) is slotted into j's unit stream, B(st4..7)
into j0's, D(st0) k<7 partials into j7's empty slots -- out-projection
(D) last.  RoPE reads the raw QK PSUM accumulator directly (fp32, and
PSUM operands are exempt from the verifier's equal-SB-base rule):
t1=(raw+b)*cos, t2f=(raw+b)*sin_signflipped, and the p^32 partner swap
(host de-interleaves W rows per head to [evens, odds]) runs as partition-
offset SBUF->SBUF DMAs on the idle SP queue -- no permutation matmuls, no
separate evacuation.  Scores are emitted as (hi=0, hi=1) row-tile pairs
(K=64, base partitions 0/64) so the PE array runs both heads concurrently
on HW.  Softmax denominators come free from the ones-column of augmented
V; reciprocals read the PSUM ones-rows directly into partitions 0 and 64
of one persistent [65,S] carrier (both legal engine start partitions,
middle rows zeroed once) so normalization broadcasts 1/den with a single
K=65 matmul per half.  The V bias is folded through the attention on the
host (softmax rows sum to 1, so attn@(V+bv) = attn@V + bv, and bvo =
Wo@bv + bo is broadcast once into a [128,D] SBUF tile and added during
the out-projection's DVE evacuation) -- neither stage B nor D carries
per-group bias matmuls.  Inputs stream on both HWDGE
queues (SP + ACT) in consumption order; elementwise work is balanced
across DVE and GPSIMD (K_EM_DVE) with ACT kept exp-only mid-stream (a
Copy/Identity op on ACT costs a ~1.3us activation-table reload vs Exp).
"""

import os
import sys

if "/opt/trn_rl_repo" not in sys.path:
    sys.path.insert(0, "/opt/trn_rl_repo")

from contextlib import ExitStack

import ml_dtypes
import numpy as np

import concourse.bass as bass
from concourse import bacc
import concourse.tile as tile
from concourse import mybir
from concourse.bass_utils import run_bass_kernel_spmd

B, S, D, H, Dh = 8, 1024, 1024, 16, 64
P = 128
NT = D // P  # 8 partition-tiles along d
ST = S // P  # 8 tiles along s/t
HF = S // 2  # 512 = matmul moving chunk / PSUM bank width (fp32)
FP = mybir.dt.float32
BF = mybir.dt.bfloat16
ROPE_BASE = 10000.0
AF = mybir.ActivationFunctionType
ALU = mybir.AluOpType

N_CORES = 8
STAGES = int(os.environ.get("K_STAGES", "5"))
# of the 32 (tt,c,hi) mask-mult units per j, how many go to DVE (rest GPSIMD)
EM_DVE = int(os.environ.get("K_EM_DVE", "16"))
CTX_LAG = int(os.environ.get("K_L", "8"))  # units ctx trails scores by

# column offsets inside the packed mega input tensor
_SIZES = [
    ("xq", NT * S), ("xk", NT * S), ("xv", ST * NT * P),
    ("wq", NT * D), ("wk", NT * D), ("wv", NT * D), ("wo", NT * D),
    ("mask", ST * S), ("cs", 2 * S), ("smalls", 24),
    ("rows", D + 3 * P),
]
OFF = {}
_o = 0
for _n, _s in _SIZES:
    OFF[_n] = _o
    _o += _s
MEGA_COLS = _o


def build_nc():
    nc = bacc.Bacc("TRN2", target_bir_lowering=False, debug=False)

    # Single mega input: every tensor packed into one [P, MEGA_COLS] bf16
    # DRAM tensor (per-input dispatch overhead through the PJRT tunnel is
    # ~70us/tensor, so one input instead of 14 dominates the bench time).
    mega = nc.dram_tensor("mega", [P, MEGA_COLS], BF, kind="ExternalInput").ap()
    xq_d = mega[:, OFF["xq"] : OFF["xq"] + NT * S]
    xk_d = mega[:, OFF["xk"] : OFF["xk"] + NT * S]
    xv_d = mega[:, OFF["xv"] : OFF["xv"] + ST * NT * P]  # st-major layout
    wq_d = mega[:, OFF["wq"] : OFF["wq"] + NT * D]
    wk_d = mega[:, OFF["wk"] : OFF["wk"] + NT * D]
    wv_d = mega[:, OFF["wv"] : OFF["wv"] + NT * D]
    wo_d = mega[:, OFF["wo"] : OFF["wo"] + NT * D]
    mask_d = mega[:, OFF["mask"] : OFF["mask"] + ST * S]
    # cs[:, 0:S] = cos table, cs[:, S:2S] = sign-folded sin table
    cs_d = mega[:, OFF["cs"] : OFF["cs"] + 2 * S]
    # smalls[:, 0:8]=u cols, 8:16=bq, 16:24=bk (de-interleaved d order)
    smalls_d = mega[:, OFF["smalls"] : OFF["smalls"] + 24]
    # rows[0, 0:D]=bvo (= Wo@bv + bo, the V-bias folded through attention
    # since softmax rows sum to 1), D:D+P=ones, then colones0, colones1
    rows_d = mega[0:1, OFF["rows"] : OFF["rows"] + D + 3 * P]
    out_d = nc.dram_tensor("out", [S, D], FP, kind="ExternalOutput").ap()

    with tile.TileContext(nc) as tc, ExitStack() as ctx:
        persist = ctx.enter_context(tc.tile_pool(name="persist", bufs=1))
        psum = ctx.enter_context(tc.tile_pool(name="ps", bufs=1, space="PSUM"))
        work = ctx.enter_context(tc.tile_pool(name="work", bufs=1))

        # ---- persistent constants / state ----
        smalls_sb = persist.tile([P, 24], BF)
        ucols = smalls_sb[:, 0:8]
        # scalar operands for the rope stt ops in fp32
        smalls_f = persist.tile([P, 16], FP)
        bqcols = smalls_f[:, 0:8]
        bkcols = smalls_f[:, 8:16]
        rows_sb = persist.tile([1, D + 3 * P], BF)
        bvorow = rows_sb[:, 0:D]
        ones_row = rows_sb[:, D : D + P]
        # [65,128] selector for the 1/den broadcast matmul: row 0 -> head 2j
        # columns, row 64 -> head 2j+1 columns, middle rows zero
        sel65 = persist.tile([65, P], BF)
        nc.gpsimd.memset(sel65[:], 0.0)
        nc.gpsimd.memset(sel65[0:1, 0:Dh], 1.0)
        nc.gpsimd.memset(sel65[64:65, Dh:P], 1.0)
        # persistent [65,S] carrier for the two reciprocal rows (partitions
        # 0 and 64 are both legal engine start partitions); middle rows
        # zeroed once so the K=65 matmul contracts them to nothing
        rec01 = persist.tile([65, S], BF)
        nc.gpsimd.memset(rec01[:], 0.0)
        # bvo broadcast to all partitions once (2 K=1 matmuls at the head)
        # so stage D adds it during evacuation instead of 16 bias matmuls
        bvo_bc = persist.tile([P, D], BF)
        cs_sb = persist.tile([P, 2 * S], BF)
        cos_sb = cs_sb[:, 0:S]
        sin_sb = cs_sb[:, S : 2 * S]
        mask_sb = persist.tile([P, ST * S], BF)

        # V augmented with ones column: [p, tt, h, 0:64]=V, [.,.,.,64]=1
        vaug = persist.tile([P, ST * H * (Dh + 1)], BF)
        vaug_v = vaug[:].rearrange("p (st h c) -> p st h c", st=ST, h=H)
        ctxu = persist.tile([P, NT * S], BF)  # ctx^T (normalized in place)
        # roped Q^T + u / roped K^T, one [P, S] tile per j (3 slots in flight)
        qbt = {}
        kbt = {}

        nc.gpsimd.memset(vaug_v[:, :, :, Dh : Dh + 1], 1.0)

        # ---- DMAs: two HWDGE queues (SP + ACT). SP streams wv/xv-rest/
        # wk/xk then j0's rope-swap chunks; ACT takes xv0 + consts + wq/xq
        # + mask so B and A(j0) start as early as possible ----

        # B-path inputs on the SP DMA queue, A-path weights in parallel on
        # the second (Activation) HWDGE queue
        pB_cm = tc.tile_pool(name="bpool", bufs=1)
        pB = pB_cm.__enter__()
        wv_sb = pB.tile([P, NT * D], BF, tag="wv")
        # chunked so B(st0)'s k-loop starts while the rest streams in
        xv_sb = pB.tile([P, ST * NT * P], BF, tag="xv")
        nc.sync.dma_start(wv_sb[:, 0 : 2 * D], wv_d[:, 0 : 2 * D])
        nc.scalar.dma_start(xv_sb[:, 0 : NT * P], xv_d[:, 0 : NT * P])
        nc.scalar.dma_start(rows_sb[:], rows_d[:])
        nc.scalar.dma_start(smalls_sb[:], smalls_d[:])
        nc.vector.tensor_copy(smalls_f[:], smalls_sb[:, 8:24])
        nc.scalar.dma_start(cs_sb[:], cs_d[:])
        for q in range(1, 4):
            nc.sync.dma_start(
                wv_sb[:, q * 2 * D : (q + 1) * 2 * D],
                wv_d[:, q * 2 * D : (q + 1) * 2 * D],
            )
        for st in range(1, 4):
            nc.sync.dma_start(
                xv_sb[:, st * NT * P : (st + 1) * NT * P],
                xv_d[:, st * NT * P : (st + 1) * NT * P],
            )
        wq_sb = work.tile([P, NT * D], BF, tag="wq")
        nc.scalar.dma_start(wq_sb[:], wq_d[:])
        xq_sb = work.tile([P, NT * S], BF, tag="xq")
        nc.scalar.dma_start(xq_sb[:], xq_d[:])
        wk_sb = work.tile([P, NT * D], BF, tag="wk")
        nc.sync.dma_start(wk_sb[:], wk_d[:])
        xk_sb = work.tile([P, NT * S], BF, tag="xk")
        nc.sync.dma_start(xk_sb[:], xk_d[:])
        # "late" pool opens after bpool closes (reuses the wv/xv SBUF);
        # emitters reach it via this cell
        late_cell = {}

        def emit_bvo_bc():
            for c in range(2):
                bb = psum.tile([P, HF], FP, tag="acc", bufs=2, name=f"bvo{c}")
                nc.tensor.matmul(
                    bb[:], ones_row, bvorow[:, c * HF : (c + 1) * HF],
                    start=True, stop=True,
                )
                nc.vector.tensor_copy(bvo_bc[:, c * HF : (c + 1) * HF], bb[:])

        # ======== stage B: V projection into vaug ========
        def emit_B(st):
            for c in range(2):
                vp = psum.tile([P, HF], FP, tag="acc", bufs=2, name=f"vp{st}_{c}")
                for k in range(NT):
                    nc.tensor.matmul(
                        vp[:],
                        xv_sb[:, st * NT * P + k * P : st * NT * P + (k + 1) * P],
                        wv_sb[:, k * D + c * HF : k * D + (c + 1) * HF],
                        start=(k == 0),
                        stop=(k == NT - 1),
                    )
                # DVE is idle during B; ACT's queue is busy issuing the
                # A-path DMAs at the head
                nc.vector.tensor_copy(
                    vaug_v[:, st, c * 8 : (c + 1) * 8, 0:Dh],
                    vp[:].rearrange("p (h c) -> p h c", h=8),
                )

        # ======== interleaved per-j stream: A (QK proj + rope) + C units ====
        # A-group g of j: g = x*2 + c with x in (q=0, k=1), c in (0, 1)
        a_state = {}

        def emit_A_raw(j, g):
            # group order (q,c0), (k,c0), (q,c1), (k,c1): the c0 tails land
            # first, which is all scores(u0/u1) need
            c, x = divmod(g, 2)
            x_sb, w_sb = (xq_sb, wq_sb) if x == 0 else (xk_sb, wk_sb)
            raw = psum.tile([P, HF], FP, tag="acc", bufs=2, name=f"raw{j}_{g}")
            for k in range(NT):
                nc.tensor.matmul(
                    raw[:],
                    w_sb[:, k * D + j * P : k * D + (j + 1) * P],
                    x_sb[:, k * S + c * HF : k * S + (c + 1) * HF],
                    start=(k == 0),
                    stop=(k == NT - 1),
                )
            a_state[(j, g)] = raw

        def emit_A_tail(j, g):
            c, x = divmod(g, 2)
            chalf = slice(c * HF, (c + 1) * HF)
            raw = a_state.pop((j, g))
            bcol = (bqcols if x == 0 else bkcols)[:, j : j + 1]
            # rope fused against the PSUM accumulator (PSUM operand is exempt
            # from the equal-SB-base-partition rule):
            #   t1[p]  = (raw[p] + b[p]) * cos[p]
            #   t2f[p] = (raw[p] + b[p]) * sinsw[p]   (sign-flipped sin table
            #            = the p^32 partner's coefficient)
            # then the p^32 partner swap runs as 4 partition-offset
            # SBUF->SBUF DMAs on the idle SP queue
            t1 = work.tile([P, HF], BF, tag="t1", bufs=2)
            nc.vector.scalar_tensor_tensor(
                t1[:], raw[:], bcol, cos_sb[:, chalf], op0=ALU.add, op1=ALU.mult
            )
            t2f = work.tile([P, HF], BF, tag="t2f", bufs=2)
            nc.vector.scalar_tensor_tensor(
                t2f[:], raw[:], bcol, sin_sb[:, chalf], op0=ALU.add, op1=ALU.mult
            )
            t2 = work.tile([P, HF], BF, tag="t2", bufs=2)
            for b in (0, 64):
                nc.sync.dma_start(t2[b : b + 32, :], t2f[b + 32 : b + 64, :])
                nc.sync.dma_start(t2[b + 32 : b + 64, :], t2f[b : b + 32, :])
            dst = qbt if x == 0 else kbt
            if j not in dst:
                dst[j] = work.tile(
                    [P, S], BF, tag=("qbt" if x == 0 else "kbt"), bufs=3,
                    name=f"{'qk'[x]}bt{j}",
                )
            dslice = dst[j][:, c * HF : (c + 1) * HF]
            if x == 0:
                # stt (u-bias scalar) only exists on DVE
                nc.vector.scalar_tensor_tensor(
                    dslice, t1[:], ucols[:, j : j + 1], t2[:],
                    op0=ALU.add, op1=ALU.add,
                )
            else:
                nc.gpsimd.tensor_tensor(dslice, t1[:], t2[:], op=ALU.add)

        # C unit u of j: u = tt*4 + c*2 + hi (hi innermost: row-tile pairs)
        em_tiles = {}
        cps_tiles = {}

        def emit_scores(j, u):
            tt, r = divmod(u, 4)
            c, hi = divmod(r, 2)
            half = hi * Dh
            sps = psum.tile([P, HF], FP, tag="sps", bufs=2, name=f"sps{j}_{u}")
            nc.tensor.matmul(
                sps[:],
                kbt[j][half : half + Dh, tt * P : (tt + 1) * P],
                qbt[j][half : half + Dh, c * HF : (c + 1) * HF],
                start=True,
                stop=True,
            )
            et = work.tile([P, HF], BF, tag="et", bufs=3)
            nc.scalar.activation(et[:], sps[:], AF.Exp, scale=0.125)
            em = work.tile([P, HF], BF, tag="em", bufs=CTX_LAG + 2)
            # spread the DVE/GPSIMD split evenly across the unit stream
            uu = u % 32
            eng = (
                nc.vector
                if (uu * EM_DVE) // 32 != ((uu + 1) * EM_DVE) // 32
                else nc.gpsimd
            )
            eng.tensor_tensor(
                em[:], et[:],
                mask_sb[:, tt * S + c * HF : tt * S + (c + 1) * HF],
                op=ALU.mult,
            )
            em_tiles[(j, u)] = em

        def emit_ctx(j, u):
            tt, r = divmod(u, 4)
            c, hi = divmod(r, 2)
            h = 2 * j + hi
            if tt == 0 and c == 0:
                cps_tiles[(j, hi)] = psum.tile(
                    [Dh + 1, S], FP, tag="cps", bufs=2, name=f"cps{j}_{hi}"
                )
            em = em_tiles.pop((j, u))
            nc.tensor.matmul(
                cps_tiles[(j, hi)][:, c * HF : (c + 1) * HF],
                vaug_v[:, tt, h, :],
                em[:],
                start=(tt == 0),
                stop=(tt == ST - 1),
            )

        rec_tiles = {}
        d_pre = {}

        def emit_C2_a(j):
            # slot-freeing half: reciprocals read the PSUM ones-row directly
            # (partition 64 -> 0), ctx evacuated to SBUF, cps banks released
            cp0 = cps_tiles.pop((j, 0))
            cp1 = cps_tiles.pop((j, 1))
            with nc.allow_low_precision(reason="bf16 1/den, ~0.4% quant"):
                nc.vector.reciprocal(rec01[0:1, :], cp0[Dh : Dh + 1, :])
                nc.vector.reciprocal(rec01[64:65, :], cp1[Dh : Dh + 1, :])
            # evacuate ctx (heads 2j -> partitions 0:64, 2j+1 -> 64:128);
            # hi=1 on ACT (Copy is table-free, no Exp thrash)
            nc.vector.tensor_copy(ctxu[0:Dh, j * S : (j + 1) * S], cp0[0:Dh, :])
            nc.scalar.copy(ctxu[Dh:P, j * S : (j + 1) * S], cp1[0:Dh, :])

        def emit_C2_b(j, rb_tag="acc"):
            # deferred half: broadcast 1/den across partitions via one K=65
            # matmul per half, then normalize in place (long dep slack by now)
            for c in range(2):
                chalf = slice(c * HF, (c + 1) * HF)
                # acc tag: A(j+1) raws are done by u26, no contention with
                # the scores stream on the sps slots
                rb = psum.tile([P, HF], FP, tag=rb_tag, bufs=2, name=f"rb{j}_{c}")
                nc.tensor.matmul(rb[:], sel65[:], rec01[:, chalf], start=True, stop=True)
                nc.vector.tensor_tensor(
                    ctxu[:, j * S + c * HF : j * S + (c + 1) * HF],
                    ctxu[:, j * S + c * HF : j * S + (c + 1) * HF],
                    rb[:],
                    op=ALU.mult,
                )

        # ---- emission schedule ----
        for st in range(4):
            emit_B(st)

        if STAGES >= 4:
            NU = 4 * ST  # 32 units per j
            # j>=1 slots: next j's projections early, C2 split around them
            A_RAW_SLOT = {10: 0, 14: 1, 18: 2, 22: 3}
            A_TAIL_SLOT = {12: 0, 16: 1, 20: 2, 24: 3}
            # j==0 slots: remaining B groups early (ctx(tt) needs vaug(tt)),
            # A(1) after
            B_SLOT_J0 = {8: 4, 12: 5, 14: 6, 16: 7}
            A_RAW_SLOT_J0 = {20: 0, 22: 1, 24: 2, 26: 3}
            A_TAIL_SLOT_J0 = {21: 0, 23: 1, 25: 2, 27: 3}
            # mask rides the ACT queue (idle after xq) so j0's rope-swap
            # DMAs get the SP queue right after xk
            for tt in range(ST):
                nc.scalar.dma_start(
                    mask_sb[:, tt * S : (tt + 1) * S],
                    mask_d[:, tt * S : (tt + 1) * S],
                )
            for g in range(4):  # j=0 A-groups up front, 1-deep pipelined
                emit_A_raw(0, g)
                if g >= 1:
                    emit_A_tail(0, g - 1)
            emit_A_tail(0, 3)
            emit_bvo_bc()
            # rest of xv after j0's swaps (needed only at j0's B slots)
            for st in range(4, ST):
                nc.sync.dma_start(
                    xv_sb[:, st * NT * P : (st + 1) * NT * P],
                    xv_d[:, st * NT * P : (st + 1) * NT * P],
                )
            for j in range(NT):
                for u in range(NU):
                    emit_scores(j, u)
                    if u >= CTX_LAG:
                        emit_ctx(j, u - CTX_LAG)
                    if j >= 1:
                        if u == 2:
                            emit_C2_a(j - 1)
                        elif u == (6 if j == NT - 1 else 26):
                            # last j: C2_b early so its acc slots free up for
                            # the D(st0) partial accumulations below
                            emit_C2_b(j - 1)
                        if j + 1 < NT:
                            if u in A_RAW_SLOT:
                                emit_A_raw(j + 1, A_RAW_SLOT[u])
                            elif u in A_TAIL_SLOT:
                                emit_A_tail(j + 1, A_TAIL_SLOT[u])
                        elif STAGES >= 5 and u in (10, 14):
                            # fill j7's empty A-slots: D(st0) k0..6 partials
                            # (ctxu j0..j6 are final by now)
                            c = 0 if u == 10 else 1
                            op = psum.tile(
                                [P, HF], FP, tag="acc", bufs=2, name=f"op0_{c}"
                            )
                            d_pre[c] = op
                            for k in range(NT - 1):
                                nc.tensor.matmul(
                                    op[:],
                                    ctxu[:, k * S : k * S + P],
                                    wo_sb[:, k * D + c * HF : k * D + (c + 1) * HF],
                                    start=(k == 0),
                                    stop=False,
                                )
                    else:
                        if u in B_SLOT_J0:
                            emit_B(B_SLOT_J0[u])
                        elif u in A_RAW_SLOT_J0:
                            emit_A_raw(1, A_RAW_SLOT_J0[u])
                        elif u in A_TAIL_SLOT_J0:
                            emit_A_tail(1, A_TAIL_SLOT_J0[u])
                for u in range(NU - CTX_LAG, NU):
                    emit_ctx(j, u)
                if j == 0:
                    # wv/xv done: free bpool, open late (reuses its SBUF)
                    pB_cm.__exit__(None, None, None)
                    late = ctx.enter_context(tc.tile_pool(name="late", bufs=1))
                    late_cell["p"] = late
                    wo_sb = late.tile([P, NT * D], BF, tag="wo")
                    nc.sync.dma_start(wo_sb[:], wo_d[:])
            emit_C2_a(NT - 1)
            # sps tag: the acc slots are held by the D(st0) partials
            emit_C2_b(NT - 1, rb_tag="sps")
        else:
            for st in range(4, ST):
                emit_B(st)
            pB_cm.__exit__(None, None, None)
            late = ctx.enter_context(tc.tile_pool(name="late", bufs=1))
            late_cell["p"] = late
            wo_sb = late.tile([P, NT * D], BF, tag="wo")
            nc.sync.dma_start(wo_sb[:], wo_d[:])

        # ---- stage D: output projection ----
        if STAGES >= 5:
            for st in range(ST):
                for c in range(2):
                    if st == 0 and c in d_pre:
                        # k0..6 already accumulated during j7's stream
                        op = d_pre.pop(c)
                        ks = range(NT - 1, NT)
                    else:
                        op = psum.tile(
                            [P, HF], FP, tag="acc", bufs=2, name=f"op{st}_{c}"
                        )
                        ks = range(NT)
                    for k in ks:
                        nc.tensor.matmul(
                            op[:],
                            ctxu[:, k * S + st * P : k * S + (st + 1) * P],
                            wo_sb[:, k * D + c * HF : k * D + (c + 1) * HF],
                            start=(k == 0),
                            stop=(k == NT - 1),
                        )
                    # DVE is idle during D; bias added during evacuation
                    ot = late.tile([P, HF], FP, tag="ot", bufs=3)
                    nc.vector.tensor_tensor(
                        ot[:], op[:], bvo_bc[:, c * HF : (c + 1) * HF], op=ALU.add
                    )
                    nc.sync.dma_start(
                        out_d[st * P : (st + 1) * P, c * HF : (c + 1) * HF], ot[:]
                    )

    nc.compile()
    return nc


def _to_sb(m):
    """[NT*P, X] -> [P, NT*X] SBUF layout (partition p holds rows p, P+p, ...)."""
    r, x = m.shape
    return np.ascontiguousarray(
        m.reshape(NT, P, x).transpose(1, 0, 2).reshape(P, NT * x)
    )


# de-interleave permutation: within each head's 64 output dims, evens first
_PERM = np.concatenate(
    [64 * h + np.concatenate([np.arange(0, 64, 2), np.arange(1, 64, 2)]) for h in range(H)]
)


def _host_consts():
    inv_freq = 1.0 / (ROPE_BASE ** (np.arange(0, Dh, 2, dtype=np.float64) / Dh))
    # de-interleaved layout: partition p (within a 64-block) holds dim
    # evens[p] for p<32, odds[p-32] for p>=32; both use freq index p%32
    freqs = np.arange(S, dtype=np.float64)[:, None] * inv_freq[None, :]  # [S, 32]
    cosf = np.cos(freqs)  # [S, 32]
    sinf = np.sin(freqs)
    costab = np.empty((P, S), np.float32)
    sintab = np.empty((P, S), np.float32)
    for p in range(P):
        f = p % 32
        costab[p, :] = cosf[:, f]
        # table holds the coefficient the p^32 PARTNER needs (t2f is computed
        # at the source partition, then DMA-swapped): partner of an even slot
        # is odd (sgn +1), partner of an odd slot is even (sgn -1)
        sgn = 1.0 if (p % 64) < 32 else -1.0
        sintab[p, :] = sgn * sinf[:, f]
    return costab, sintab


_CONSTS = {}


def host_in_maps(query, key, value, mask, Wq, bq, Wk, bk, Wv, bv, u_bias, Wo, bo):
    bf = ml_dtypes.bfloat16
    if not _CONSTS:
        costab, sintab = _host_consts()
        _CONSTS["cs"] = np.concatenate([costab, sintab], axis=1)
    u = np.asarray(u_bias, np.float32).reshape(H * Dh)[_PERM].reshape(H, Dh)
    bq_p = np.asarray(bq, np.float32)[_PERM]
    bk_p = np.asarray(bk, np.float32)[_PERM]
    smalls = np.zeros((P, 24), np.float32)
    for j in range(NT):
        smalls[:, j] = np.concatenate([u[2 * j], u[2 * j + 1]])
    smalls[:, 8:16] = bq_p.reshape(NT, P).T
    smalls[:, 16:24] = bk_p.reshape(NT, P).T
    rows = np.zeros((P, D + 3 * P), np.float32)
    colones0 = np.zeros(P, np.float32)
    colones0[0:Dh] = 1.0
    colones1 = np.zeros(P, np.float32)
    colones1[Dh:P] = 1.0
    # attn rows sum to 1, so attn@(V+bv) = attn@V + bv; fold bv through the
    # out projection: bvo = Wo@bv + bo
    bvo = np.asarray(Wo, np.float32) @ np.asarray(bv, np.float32) + np.asarray(
        bo, np.float32
    )
    rows[0] = np.concatenate([bvo, np.ones(P, np.float32), colones0, colones1])

    def pack(pieces):
        m = np.empty((P, MEGA_COLS), bf)
        for name, arr in pieces.items():
            o = OFF[name]
            m[:, o : o + arr.shape[1]] = arr.astype(bf)
        return m

    def xv_layout(v):
        # [P, k*S + s] -> [P, st*(NT*P) + k*P + ss] (st-major)
        a = _to_sb(v)  # [P, NT*S]
        a = a.reshape(P, NT, ST, P).transpose(0, 2, 1, 3).reshape(P, ST * NT * P)
        return np.ascontiguousarray(a)

    shared = dict(
        wq=_to_sb(np.asarray(Wq, np.float32).T[:, _PERM].astype(bf)),
        wk=_to_sb(np.asarray(Wk, np.float32).T[:, _PERM].astype(bf)),
        wv=_to_sb(np.asarray(Wv, np.float32).T.astype(bf)),
        wo=_to_sb(np.asarray(Wo, np.float32).T.astype(bf)),
        cs=_CONSTS["cs"],
        smalls=smalls,
        rows=rows,
    )
    in_maps = []
    for b in range(N_CORES):
        pieces = dict(shared)
        pieces["xq"] = _to_sb(np.asarray(query[b], np.float32).T.astype(bf))
        pieces["xk"] = _to_sb(np.asarray(key[b], np.float32).T.astype(bf))
        pieces["xv"] = xv_layout(np.asarray(value[b], np.float32).T.astype(bf))
        pieces["mask"] = _to_sb((~np.asarray(mask[b], bool)).T.astype(bf))
        in_maps.append(dict(mega=pack(pieces)))
    return in_maps


_CACHED = {}


def kernel(query, key, value, mask, Wq, bq, Wk, bk, Wv, bv, u_bias, Wo, bo):
    if "nc" not in _CACHED:
        _CACHED["nc"] = build_nc()
    nc = _CACHED["nc"]
    in_maps = host_in_maps(
        query, key, value, mask, Wq, bq, Wk, bk, Wv, bv, u_bias, Wo, bo
    )
    res = run_bass_kernel_spmd(nc, in_maps, list(range(N_CORES)))
    return np.stack([res.results[b]["out"] for b in range(N_CORES)], axis=0)
